# revision 24
# baseline (speedup 1.0000x reference)
"""StyleGAN2-style modulated conv (per-sample 3x3, 256->256 ch, 128x128) on 8 TRN2 cores.

Data-parallel over batch: core c computes sample c entirely on-chip.
1D Winograd F(2,3) along H cuts PE work 1.5x vs direct shift-and-matmul:
per output row-pair only 4 H-taps x 3 W-taps of N=512 matmuls (vs 2x9).
Row transforms (single adds/subs of adjacent rows) run on DVE; weight
transform + style modulation on GpSimd; casts + demodulation on ACT.
"""

import numpy as np
from contextlib import ExitStack

import concourse.bass as bass
import concourse.mybir as mybir
import concourse.tile as tile
from concourse import bacc
from concourse.masks import make_identity

FP32 = mybir.dt.float32
FP16 = mybir.dt.float16
AX = mybir.AxisListType
AF = mybir.ActivationFunctionType
OP = mybir.AluOpType

B = 8
CI = 256
CO = 256
H = 128
W = 128
KS = 3
Z = 512
NKK = KS * KS          # 9 kernel taps
IT = CI // 128         # 2 input-channel tiles
OT = CO // 128         # 2 output-channel tiles
RG = 16                # output rows per group
G = H // RG            # 8 row groups
NP = RG // 2           # 8 winograd row-pairs per group
WP = W + 2             # padded width (zero cols at 0 and WP-1)
TH = 4                 # winograd taps along H
EPS = 1e-8


def build_nc() -> bass.Bass:
    nc = bacc.Bacc("TRN2", target_bir_lowering=False, debug=False)
    x_d = nc.dram_tensor("x", [CI, H, W], FP32, kind="ExternalInput")
    w_d = nc.dram_tensor("w", [Z], FP32, kind="ExternalInput")
    wt_d = nc.dram_tensor("weight", [CO, CI, KS, KS], FP32, kind="ExternalInput")
    aw_d = nc.dram_tensor("affine_w", [CI, Z], FP32, kind="ExternalInput")
    ab_d = nc.dram_tensor("affine_b", [CI], FP32, kind="ExternalInput")
    y_d = nc.dram_tensor("y", [CO, H, W], FP32, kind="ExternalOutput")

    with tile.TileContext(nc) as tc, ExitStack() as ctx:
        singles = ctx.enter_context(tc.tile_pool(name="singles", bufs=1))
        work = ctx.enter_context(tc.tile_pool(name="work", bufs=1))
        wbuild = ctx.enter_context(tc.tile_pool(name="wbuild", bufs=1))
        xstage = ctx.enter_context(tc.tile_pool(name="xstage", bufs=3))
        xpool = ctx.enter_context(tc.tile_pool(name="xg", bufs=4))
        xtpool = ctx.enter_context(tc.tile_pool(name="xt", bufs=6))

        zrow = singles.tile([128, WP], FP16)
        nc.vector.memset(zrow, 0.0)
        ident = singles.tile([128, 128], FP32)
        make_identity(nc, ident)
        eps_t = singles.tile([128, 1], FP32)
        nc.vector.memset(eps_t, EPS)

        # ---- weight DMA first, split by i-tile half ([O, I*9] contiguous).
        # wo pool is scoped: its SBUF is reused by the og/escr pools below.
        wo_ctx = tc.tile_pool(name="wo", bufs=2)
        wopool = wo_ctx.__enter__()
        wo = [
            wopool.tile([128, CI * NKK], FP32, name=f"wo{ot}", tag=f"wo{ot}")
            for ot in range(OT)
        ]
        HALF = (CI // IT) * NKK  # 1152 elements per i-half

        def load_wo_half(it):
            for ot in range(OT):
                nc.sync.dma_start(
                    out=wo[ot][:, it * HALF:(it + 1) * HALF],
                    in_=wt_d[
                        ot * 128:(ot + 1) * 128, it * 128:(it + 1) * 128
                    ].rearrange("o i kh kw -> o (i kh kw)"),
                )

        load_wo_half(0)
        load_wo_half(1)

        # ---- small input DMAs (style path) ----
        wb = singles.tile([128, Z], FP32)
        w_ap = w_d[:]
        nc.sync.dma_start(
            out=wb,
            in_=bass.AP(tensor=w_ap.tensor, offset=w_ap.offset, ap=[[0, 128], [1, Z]]),
        )
        af, ab1 = [], []
        for it in range(IT):
            a = singles.tile([128, Z], FP32, tag=f"af{it}")
            nc.sync.dma_start(out=a, in_=aw_d[it * 128:(it + 1) * 128, :])
            af.append(a)
            abt = singles.tile([128, 1], FP32, tag=f"ab{it}")
            nc.sync.dma_start(
                out=abt, in_=ab_d[it * 128:(it + 1) * 128].rearrange("(p o) -> p o", o=1)
            )
            ab1.append(abt)

        # ---- x row-group loads: DMA fp32 stage; cast on ACT; pads on DVE ----
        xg_tiles: dict = {}
        xt_tiles: dict = {}

        def lg_dma(g: int):
            r0 = g * RG
            lo, hi = r0 - 1, r0 + RG + 1
            clo, chi = max(lo, 0), min(hi, H)
            nrows = chi - clo
            gs = []
            for it in range(IT):
                stg = xstage.tile([128, RG + 2, W], FP32, name="stg", tag="stg")
                nc.sync.dma_start(
                    out=stg[:, 0:nrows, :],
                    in_=x_d[it * 128:(it + 1) * 128, clo:chi, :],
                )
                gs.append(stg)
            xg_tiles[g] = (gs, lo, hi, clo, chi)

        def lg_cast(g: int):
            gs, lo, hi, clo, chi = xg_tiles[g]
            nrows = chi - clo
            gx = []
            for it in range(IT):
                t = xpool.tile([128, RG + 2, WP], FP16, name="xg", tag="xg")
                # fp32 -> fp16 cast on ACT; zero pads on GpSimd (DVE stays
                # clear for the weight-transpose copies and PSUM evictions)
                nc.scalar.copy(
                    out=t[:, clo - lo: chi - lo, 1:W + 1], in_=gs[it][:, 0:nrows, :]
                )
                nc.gpsimd.tensor_copy(out=t[:, :, 0], in_=zrow[:, 0:RG + 2])
                nc.gpsimd.tensor_copy(out=t[:, :, WP - 1], in_=zrow[:, 0:RG + 2])
                if lo < 0:
                    nc.gpsimd.tensor_copy(out=t[:, 0, :], in_=zrow)
                if hi > H:
                    nc.gpsimd.tensor_copy(out=t[:, RG + 1, :], in_=zrow)
                gx.append(t)
            xg_tiles[g] = gx
            xt_tiles[g] = []

        def transform_it(g: int, it: int):
            # winograd F(2,3) row transform: pair p covers output rows
            # (2p, 2p+1); d0..d3 = padded input rows 2p-1 .. 2p+2 = xg[2p..2p+3]
            xg = xg_tiles[g][it]
            xt = xtpool.tile([128, TH, NP, WP], FP16, name="xt", tag="xt")
            xt_tiles[g].append(xt)
            ev = xg.rearrange("p (r c) w -> p r c w", c=2)  # [128, 9, 2, WP]
            d0 = ev[:, 0:NP, 0, :]   # rows 0,2,..,14
            d1 = ev[:, 0:NP, 1, :]   # rows 1,3,..,15
            d2 = ev[:, 1:NP + 1, 0, :]  # rows 2,4,..,16
            d3 = ev[:, 1:NP + 1, 1, :]  # rows 3,5,..,17
            nc.gpsimd.tensor_sub(xt[:, 0], d0, d2)
            nc.gpsimd.tensor_add(xt[:, 1], d1, d2)
            nc.gpsimd.tensor_sub(xt[:, 2], d1, d2)
            nc.gpsimd.tensor_sub(xt[:, 3], d1, d3)

        lg_dma(0)
        lg_dma(1)
        lg_dma(2)

        # ---- PE transpose: wT[it][i, kk*CO+o] = weight[o, i, kk] ----
        # it0 PSUM->SBUF copies on DVE, it1 copies on ACT (idle until casts),
        # so neither engine's FIFO head-of-line blocks the other's path.
        wT = [
            singles.tile([128, NKK * CO], FP32, name=f"wT{it}", tag=f"wT{it}")
            for it in range(IT)
        ]
        # winograd-transformed modulated weights, slab t*3*CO + kw*CO + o
        wTs = [
            singles.tile([128, TH * KS * CO], FP16, name=f"wTs{it}", tag=f"wTs{it}")
            for it in range(IT)
        ]
        with tc.tile_pool(name="tpsum", bufs=6, space="PSUM") as tps:
            for it in range(IT):
                ceng = nc.vector if it == 0 else nc.scalar
                for ot in range(OT):
                    for kk in range(NKK):
                        pt = tps.tile([128, 128], FP32, name="pt", tag="pt")
                        src = wo[ot].rearrange("o (i k) -> o i k", k=NKK)[
                            :, it * 128:(it + 1) * 128, kk
                        ]
                        nc.tensor.transpose(out=pt, in_=src, identity=ident)
                        if it == 0:
                            nc.vector.tensor_copy(
                                out=wT[it][:, kk * CO + ot * 128:
                                           kk * CO + (ot + 1) * 128],
                                in_=pt,
                            )
                        else:
                            nc.scalar.copy(
                                out=wT[it][:, kk * CO + ot * 128:
                                           kk * CO + (ot + 1) * 128],
                                in_=pt,
                            )

        wo_ctx.__exit__(None, None, None)
        lg_cast(0)

        # ---- style + winograd weight build (all DVE) ----
        st, sth, st2 = [], [], []
        KH = KS * CO

        def style_and_wbuild(it):
            stt = work.tile([128, Z], FP32, name="stt", tag="styletmp")
            nc.vector.tensor_mul(stt, af[it], wb)
            s = singles.tile([128, 1], FP32, name="s", tag=f"st{it}")
            nc.vector.reduce_sum(s, stt, axis=AX.X)
            nc.vector.tensor_add(s, s, ab1[it])
            nc.vector.tensor_scalar_add(s, s, 1.0)
            st.append(s)
            sh = singles.tile([128, 1], FP32, name="sh", tag=f"sth{it}")
            nc.vector.tensor_scalar_mul(sh, s, 0.5)
            sth.append(sh)
            s2 = singles.tile([128, 1], FP32, name="s2", tag=f"st2{it}")
            nc.vector.tensor_mul(s2, s, s)
            st2.append(s2)
            k0 = wT[it][:, 0 * KH:1 * KH]
            k1 = wT[it][:, 1 * KH:2 * KH]
            k2 = wT[it][:, 2 * KH:3 * KH]
            u1 = wbuild.tile([128, KH], FP32, name="u1", tag="u1")
            aa = wbuild.tile([128, KH], FP32, name="aa", tag="aa")
            bb = wbuild.tile([128, KH], FP32, name="bb", tag="bb")
            nc.vector.tensor_add(u1, k0, k2)
            nc.vector.tensor_add(aa, u1, k1)
            nc.vector.tensor_sub(bb, k1, u1)
            ws = wTs[it]
            nc.vector.tensor_scalar_mul(ws[:, 0 * KH:1 * KH], k0, s)
            nc.vector.tensor_scalar_mul(ws[:, 1 * KH:2 * KH], aa, sh)
            nc.vector.tensor_scalar_mul(ws[:, 2 * KH:3 * KH], bb, sh)
            nc.vector.tensor_scalar_mul(ws[:, 3 * KH:4 * KH], k2, s)

        style_and_wbuild(0)
        transform_it(0, 0)
        style_and_wbuild(1)
        transform_it(0, 1)

        # wsq[it][i, o] = sum_kk wT[i,kk,o]^2 (DVE square + add chain)
        wsq = []
        for it in range(IT):
            q = singles.tile([128, CO], FP32, name="wsq", tag=f"wsq{it}")
            tmp = wbuild.tile([128, CO], FP32, name="wsqt", tag="wsqt")
            nc.vector.tensor_mul(q, wT[it][:, 0:CO], wT[it][:, 0:CO])
            for kk in range(1, NKK):
                slab = wT[it][:, kk * CO:(kk + 1) * CO]
                nc.vector.tensor_mul(tmp, slab, slab)
                nc.vector.tensor_add(q, q, tmp)
            wsq.append(q)

        lg_cast(1)
        transform_it(1, 0)
        transform_it(1, 1)

        # ---- conv: winograd row-pairs, 4 PSUM planes per (g, ot, half) ----
        # PSUM budget: cpsum 7 banks (4 per block + 3 of the next) + dpsum 1.
        opool = ctx.enter_context(tc.tile_pool(name="og", bufs=3))
        spool = ctx.enter_context(tc.tile_pool(name="escr", bufs=2))
        cpsum = ctx.enter_context(tc.tile_pool(name="cpsum", bufs=7, space="PSUM"))
        dpsum = ctx.enter_context(tc.tile_pool(name="dpsum", bufs=1, space="PSUM"))
        dn = []

        def emit_denom():
            # dn[ot] = rsqrt(wsq[:, ot-block].T @ style^2 + eps) as an O-column
            for ot in range(OT):
                pd = dpsum.tile([128, 1], FP32, name="pd", tag="pd")
                for it in range(IT):
                    nc.tensor.matmul(
                        pd,
                        lhsT=wsq[it][:, ot * 128:(ot + 1) * 128],
                        rhs=st2[it],
                        start=(it == 0),
                        stop=(it == IT - 1),
                    )
                dcol = singles.tile([128, 1], FP32, name="dn", tag=f"dn{ot}")
                nc.scalar.activation(out=dcol, in_=pd, func=AF.Sqrt, bias=eps_t)
                nc.vector.reciprocal(dcol, dcol)
                dn.append(dcol)

        KH = KS * CO

        def half_matmuls(g: int, ot: int, h: int):
            P = [
                cpsum.tile([128, 512], FP32, name="pg", tag="pg")
                for _ in range(TH)
            ]
            for it in range(IT):
                xt = xt_tiles[g][it]
                for t in range(TH):
                    for kw in range(KS):
                        nc.tensor.matmul(
                            P[t],
                            lhsT=wTs[it][:, t * KH + kw * CO + ot * 128:
                                         t * KH + kw * CO + ot * 128 + 128],
                            rhs=xt[:, t, 4 * h:4 * h + 4, kw:kw + W],
                            start=(it == 0 and kw == 0),
                            stop=(it == IT - 1 and kw == KS - 1),
                        )
            return P

        def evict_dve(P):
            # y0 = (P0+P1)+P2, y1 = (P1-P2)-P3; only one PSUM operand per
            # DVE op is legal, so P1/P3 go through ACT copies to SBUF, and
            # the final all-SBUF op runs on GpSimd. Frees banks, no dn wait.
            Pv = [p.rearrange("p (a b) -> p a b", b=W) for p in P]
            og = opool.tile([128, 2 * TH, W], FP32, name="og", tag="og")
            ogv = og.rearrange("p (a j) w -> p a j w", j=2)
            c1 = spool.tile([128, TH, W], FP32, name="c1", tag="c1")
            c3 = spool.tile([128, TH, W], FP32, name="c3", tag="c3")
            u = spool.tile([128, TH, W], FP32, name="u", tag="u")
            v = spool.tile([128, TH, W], FP32, name="v", tag="v")
            nc.scalar.copy(out=c1, in_=Pv[1])
            nc.scalar.copy(out=c3, in_=Pv[3])
            nc.vector.tensor_add(u, c1, Pv[0])
            nc.vector.tensor_add(ogv[:, :, 0, :], u, Pv[2])
            nc.vector.tensor_sub(v, c1, Pv[2])
            nc.gpsimd.tensor_sub(ogv[:, :, 1, :], v, c3)
            return og

        def evict_fin(og, g: int, ot: int, h: int):
            # demodulation scale (in-place, ACT) + store
            nc.scalar.mul(out=og, in_=og, mul=dn[ot])
            nc.sync.dma_start(
                out=y_d[ot * 128:(ot + 1) * 128,
                        g * RG + h * (RG // 2): g * RG + (h + 1) * (RG // 2), :],
                in_=og,
            )

        def block(g: int, ot: int, h: int):
            P = half_matmuls(g, ot, h)
            og = evict_dve(P)
            evict_fin(og, g, ot, h)

        # group 0: first two blocks' matmuls + bank-freeing DVE evictions,
        # then denom (wsq ready by then), then the deferred ACT demod+store
        # (which waits on dn — emitted after sqrt to keep ACT's FIFO acyclic).
        p00 = half_matmuls(0, 0, 0)
        og00 = evict_dve(p00)
        p01 = half_matmuls(0, 0, 1)
        og01 = evict_dve(p01)
        emit_denom()
        evict_fin(og00, 0, 0, 0)
        evict_fin(og01, 0, 0, 1)
        lg_cast(2)
        for h in range(2):
            block(0, 1, h)

        for g in range(1, G):
            if g + 2 < G:
                lg_dma(g + 2)
            if g + 1 < G:
                for it in range(IT):
                    transform_it(g + 1, it)
            block(g, 0, 0)
            block(g, 0, 1)
            block(g, 1, 0)
            if g + 2 < G:
                lg_cast(g + 2)
            block(g, 1, 1)
    nc.finalize()
    return nc


_CACHE: dict = {}


def _get_nc() -> bass.Bass:
    if "nc" not in _CACHE:
        _CACHE["nc"] = build_nc()
    return _CACHE["nc"]


def make_in_maps(x, w, weight, affine_w, affine_b):
    x = np.ascontiguousarray(x, dtype=np.float32)
    w = np.ascontiguousarray(w, dtype=np.float32)
    weight = np.ascontiguousarray(weight, dtype=np.float32)
    affine_w = np.ascontiguousarray(affine_w, dtype=np.float32)
    affine_b = np.ascontiguousarray(affine_b, dtype=np.float32)
    return [
        {
            "x": x[c],
            "w": w[c],
            "weight": weight,
            "affine_w": affine_w,
            "affine_b": affine_b,
        }
        for c in range(B)
    ]


def run_on_hw(inputs: dict, trace: bool = False, tmpdir: str | None = None):
    from concourse.bass_utils import run_bass_kernel_spmd

    nc = _get_nc()
    in_maps = make_in_maps(**inputs)
    res = run_bass_kernel_spmd(
        nc, in_maps, core_ids=list(range(B)), trace=trace, tmpdir=tmpdir
    )
    y = np.stack([r["y"] for r in res.results], axis=0)
    return y, res


def kernel(x, w, weight, affine_w, affine_b):
    y, _ = run_on_hw(
        dict(x=x, w=w, weight=weight, affine_w=affine_w, affine_b=affine_b)
    )
    return y


# revision 28
# speedup vs baseline: 1.3097x; 1.3097x over previous
"""StyleGAN2-style modulated conv (per-sample 3x3, 256->256 ch, 128x128) on 8 TRN2 cores.

Data-parallel over batch: core c computes sample c entirely on-chip.
1D Winograd F(2,3) along H cuts PE work 1.5x vs direct shift-and-matmul:
per output row-pair only 4 H-taps x 3 W-taps of N=512 matmuls (vs 2x9).
Row transforms (single adds/subs of adjacent rows) run on DVE; weight
transform + style modulation on GpSimd; casts + demodulation on ACT.
"""

import numpy as np
from contextlib import ExitStack

import concourse.bass as bass
import concourse.mybir as mybir
import concourse.tile as tile
from concourse import bacc
from concourse.masks import make_identity

FP32 = mybir.dt.float32
FP16 = mybir.dt.float16
AX = mybir.AxisListType
AF = mybir.ActivationFunctionType
OP = mybir.AluOpType

B = 8
CI = 256
CO = 256
H = 128
W = 128
KS = 3
Z = 512
NKK = KS * KS          # 9 kernel taps
IT = CI // 128         # 2 input-channel tiles
OT = CO // 128         # 2 output-channel tiles
RG = 16                # output rows per group
G = H // RG            # 8 row groups
NP = RG // 2           # 8 winograd row-pairs per group
WP = W + 2             # padded width (zero cols at 0 and WP-1)
TH = 4                 # winograd taps along H
EPS = 1e-8


def build_nc() -> bass.Bass:
    nc = bacc.Bacc("TRN2", target_bir_lowering=False, debug=False)
    x_d = nc.dram_tensor("x", [CI, H, W], FP32, kind="ExternalInput")
    w_d = nc.dram_tensor("w", [Z], FP32, kind="ExternalInput")
    wt_d = nc.dram_tensor("weight", [CO, CI, KS, KS], FP32, kind="ExternalInput")
    aw_d = nc.dram_tensor("affine_w", [CI, Z], FP32, kind="ExternalInput")
    ab_d = nc.dram_tensor("affine_b", [CI], FP32, kind="ExternalInput")
    y_d = nc.dram_tensor("y", [CO, H, W], FP32, kind="ExternalOutput")

    with tile.TileContext(nc) as tc, ExitStack() as ctx:
        singles = ctx.enter_context(tc.tile_pool(name="singles", bufs=1))
        work = ctx.enter_context(tc.tile_pool(name="work", bufs=1))
        wbuild = ctx.enter_context(tc.tile_pool(name="wbuild", bufs=1))
        xstage = ctx.enter_context(tc.tile_pool(name="xstage", bufs=3))
        xpool = ctx.enter_context(tc.tile_pool(name="xg", bufs=4))
        xtpool = ctx.enter_context(tc.tile_pool(name="xt", bufs=6))

        zrow = singles.tile([128, WP], FP16)
        nc.vector.memset(zrow, 0.0)
        ident = singles.tile([128, 128], FP32)
        make_identity(nc, ident)
        eps_t = singles.tile([128, 1], FP32)
        nc.vector.memset(eps_t, EPS)

        # ---- weight DMA first, split by i-tile half ([O, I*9] contiguous).
        # wo pool is scoped: its SBUF is reused by the og/escr pools below.
        wo_ctx = tc.tile_pool(name="wo", bufs=2)
        wopool = wo_ctx.__enter__()
        wo = [
            wopool.tile([128, CI * NKK], FP32, name=f"wo{ot}", tag=f"wo{ot}")
            for ot in range(OT)
        ]
        HALF = (CI // IT) * NKK  # 1152 elements per i-half

        def load_wo_half(it):
            for ot in range(OT):
                nc.sync.dma_start(
                    out=wo[ot][:, it * HALF:(it + 1) * HALF],
                    in_=wt_d[
                        ot * 128:(ot + 1) * 128, it * 128:(it + 1) * 128
                    ].rearrange("o i kh kw -> o (i kh kw)"),
                )

        load_wo_half(0)
        load_wo_half(1)

        # ---- small input DMAs (style path) ----
        wb = singles.tile([128, Z], FP32)
        w_ap = w_d[:]
        nc.sync.dma_start(
            out=wb,
            in_=bass.AP(tensor=w_ap.tensor, offset=w_ap.offset, ap=[[0, 128], [1, Z]]),
        )
        af, ab1 = [], []
        for it in range(IT):
            a = singles.tile([128, Z], FP32, tag=f"af{it}")
            nc.sync.dma_start(out=a, in_=aw_d[it * 128:(it + 1) * 128, :])
            af.append(a)
            abt = singles.tile([128, 1], FP32, tag=f"ab{it}")
            nc.sync.dma_start(
                out=abt, in_=ab_d[it * 128:(it + 1) * 128].rearrange("(p o) -> p o", o=1)
            )
            ab1.append(abt)

        # ---- x row-group loads: DMA fp32 stage; cast on ACT; pads on DVE ----
        xg_tiles: dict = {}
        xt_tiles: dict = {}

        def lg_dma(g: int):
            r0 = g * RG
            lo, hi = r0 - 1, r0 + RG + 1
            clo, chi = max(lo, 0), min(hi, H)
            nrows = chi - clo
            gs = []
            for it in range(IT):
                stg = xstage.tile([128, RG + 2, W], FP32, name="stg", tag="stg")
                nc.sync.dma_start(
                    out=stg[:, 0:nrows, :],
                    in_=x_d[it * 128:(it + 1) * 128, clo:chi, :],
                )
                gs.append(stg)
            xg_tiles[g] = (gs, lo, hi, clo, chi)

        def lg_cast(g: int):
            gs, lo, hi, clo, chi = xg_tiles[g]
            nrows = chi - clo
            gx = []
            for it in range(IT):
                t = xpool.tile([128, RG + 2, WP], FP16, name="xg", tag="xg")
                # fp32 -> fp16 cast on ACT; zero pads on GpSimd (DVE stays
                # clear for the weight-transpose copies and PSUM evictions)
                nc.scalar.copy(
                    out=t[:, clo - lo: chi - lo, 1:W + 1], in_=gs[it][:, 0:nrows, :]
                )
                nc.gpsimd.tensor_copy(out=t[:, :, 0], in_=zrow[:, 0:RG + 2])
                nc.gpsimd.tensor_copy(out=t[:, :, WP - 1], in_=zrow[:, 0:RG + 2])
                if lo < 0:
                    nc.gpsimd.tensor_copy(out=t[:, 0, :], in_=zrow)
                if hi > H:
                    nc.gpsimd.tensor_copy(out=t[:, RG + 1, :], in_=zrow)
                gx.append(t)
            xg_tiles[g] = gx
            xt_tiles[g] = []

        def transform_it(g: int, it: int):
            # winograd F(2,3) row transform: pair p covers output rows
            # (2p, 2p+1); d0..d3 = padded input rows 2p-1 .. 2p+2 = xg[2p..2p+3]
            xg = xg_tiles[g][it]
            xt = xtpool.tile([128, TH, NP, WP], FP16, name="xt", tag="xt")
            xt_tiles[g].append(xt)
            ev = xg.rearrange("p (r c) w -> p r c w", c=2)  # [128, 9, 2, WP]
            d0 = ev[:, 0:NP, 0, :]   # rows 0,2,..,14
            d1 = ev[:, 0:NP, 1, :]   # rows 1,3,..,15
            d2 = ev[:, 1:NP + 1, 0, :]  # rows 2,4,..,16
            d3 = ev[:, 1:NP + 1, 1, :]  # rows 3,5,..,17
            nc.vector.tensor_sub(xt[:, 0], d0, d2)
            nc.vector.tensor_add(xt[:, 1], d1, d2)
            nc.vector.tensor_sub(xt[:, 2], d1, d2)
            nc.vector.tensor_sub(xt[:, 3], d1, d3)

        lg_dma(0)
        lg_dma(1)
        lg_dma(2)

        # ---- PE transpose: wT[it][i, kk*CO+o] = weight[o, i, kk] ----
        # it0 PSUM->SBUF copies on DVE, it1 copies on ACT (idle until casts),
        # so neither engine's FIFO head-of-line blocks the other's path.
        wT = [
            singles.tile([128, NKK * CO], FP32, name=f"wT{it}", tag=f"wT{it}")
            for it in range(IT)
        ]
        # winograd-transformed modulated weights, slab t*3*CO + kw*CO + o
        wTs = [
            singles.tile([128, TH * KS * CO], FP16, name=f"wTs{it}", tag=f"wTs{it}")
            for it in range(IT)
        ]
        with tc.tile_pool(name="tpsum", bufs=6, space="PSUM") as tps:
            for it in range(IT):
                ceng = nc.vector if it == 0 else nc.scalar
                for ot in range(OT):
                    for kk in range(NKK):
                        pt = tps.tile([128, 128], FP32, name="pt", tag="pt")
                        src = wo[ot].rearrange("o (i k) -> o i k", k=NKK)[
                            :, it * 128:(it + 1) * 128, kk
                        ]
                        nc.tensor.transpose(out=pt, in_=src, identity=ident)
                        if it == 0:
                            nc.vector.tensor_copy(
                                out=wT[it][:, kk * CO + ot * 128:
                                           kk * CO + (ot + 1) * 128],
                                in_=pt,
                            )
                        else:
                            nc.scalar.copy(
                                out=wT[it][:, kk * CO + ot * 128:
                                           kk * CO + (ot + 1) * 128],
                                in_=pt,
                            )

        wo_ctx.__exit__(None, None, None)
        lg_cast(0)

        # ---- style + winograd weight build (all DVE) ----
        st, sth, st2 = [], [], []
        KH = KS * CO

        def style_and_wbuild(it):
            stt = work.tile([128, Z], FP32, name="stt", tag="styletmp")
            nc.vector.tensor_mul(stt, af[it], wb)
            s = singles.tile([128, 1], FP32, name="s", tag=f"st{it}")
            nc.vector.reduce_sum(s, stt, axis=AX.X)
            nc.vector.tensor_add(s, s, ab1[it])
            nc.vector.tensor_scalar_add(s, s, 1.0)
            st.append(s)
            sh = singles.tile([128, 1], FP32, name="sh", tag=f"sth{it}")
            nc.vector.tensor_scalar_mul(sh, s, 0.5)
            sth.append(sh)
            s2 = singles.tile([128, 1], FP32, name="s2", tag=f"st2{it}")
            nc.vector.tensor_mul(s2, s, s)
            st2.append(s2)
            k0 = wT[it][:, 0 * KH:1 * KH]
            k1 = wT[it][:, 1 * KH:2 * KH]
            k2 = wT[it][:, 2 * KH:3 * KH]
            u1 = wbuild.tile([128, KH], FP32, name="u1", tag="u1")
            aa = wbuild.tile([128, KH], FP32, name="aa", tag="aa")
            bb = wbuild.tile([128, KH], FP32, name="bb", tag="bb")
            nc.vector.tensor_add(u1, k0, k2)
            nc.vector.tensor_add(aa, u1, k1)
            nc.vector.tensor_sub(bb, k1, u1)
            ws = wTs[it]
            nc.vector.tensor_scalar_mul(ws[:, 0 * KH:1 * KH], k0, s)
            nc.vector.tensor_scalar_mul(ws[:, 1 * KH:2 * KH], aa, sh)
            nc.vector.tensor_scalar_mul(ws[:, 2 * KH:3 * KH], bb, sh)
            nc.vector.tensor_scalar_mul(ws[:, 3 * KH:4 * KH], k2, s)

        style_and_wbuild(0)
        transform_it(0, 0)
        style_and_wbuild(1)
        transform_it(0, 1)

        # wsq[it][i, o] = sum_kk wT[i,kk,o]^2: square+add chain, it0 on DVE
        # and it1 on GpSimd (off the critical x-transform path on both)
        wsq = []
        for it in range(IT):
            eng = nc.vector if it == 0 else nc.gpsimd
            q = singles.tile([128, CO], FP32, name="wsq", tag=f"wsq{it}")
            tmp = wbuild.tile([128, CO], FP32, name="wsqt", tag=f"wsqt{it}")
            eng.tensor_mul(q, wT[it][:, 0:CO], wT[it][:, 0:CO])
            for kk in range(1, NKK):
                slab = wT[it][:, kk * CO:(kk + 1) * CO]
                eng.tensor_mul(tmp, slab, slab)
                eng.tensor_add(q, q, tmp)
            wsq.append(q)

        lg_cast(1)
        transform_it(1, 0)
        transform_it(1, 1)

        # ---- conv: winograd row-pairs, 4 PSUM planes per (g, ot, half) ----
        # PSUM budget: cpsum 7 banks (4 per block + 3 of the next) + dpsum 1.
        opool = ctx.enter_context(tc.tile_pool(name="og", bufs=3))
        spool = ctx.enter_context(tc.tile_pool(name="escr", bufs=2))
        cpsum = ctx.enter_context(tc.tile_pool(name="cpsum", bufs=7, space="PSUM"))
        dpsum = ctx.enter_context(tc.tile_pool(name="dpsum", bufs=1, space="PSUM"))
        dn = []

        def emit_denom():
            # dn[ot] = rsqrt(wsq[:, ot-block].T @ style^2 + eps) as an O-column
            for ot in range(OT):
                pd = dpsum.tile([128, 1], FP32, name="pd", tag="pd")
                for it in range(IT):
                    nc.tensor.matmul(
                        pd,
                        lhsT=wsq[it][:, ot * 128:(ot + 1) * 128],
                        rhs=st2[it],
                        start=(it == 0),
                        stop=(it == IT - 1),
                    )
                dcol = singles.tile([128, 1], FP32, name="dn", tag=f"dn{ot}")
                nc.scalar.activation(out=dcol, in_=pd, func=AF.Sqrt, bias=eps_t)
                nc.vector.reciprocal(dcol, dcol)
                dn.append(dcol)

        KH = KS * CO

        def half_matmuls(g: int, ot: int, h: int):
            P = [
                cpsum.tile([128, 512], FP32, name="pg", tag="pg")
                for _ in range(TH)
            ]
            for it in range(IT):
                xt = xt_tiles[g][it]
                for t in range(TH):
                    for kw in range(KS):
                        nc.tensor.matmul(
                            P[t],
                            lhsT=wTs[it][:, t * KH + kw * CO + ot * 128:
                                         t * KH + kw * CO + ot * 128 + 128],
                            rhs=xt[:, t, 4 * h:4 * h + 4, kw:kw + W],
                            start=(it == 0 and kw == 0),
                            stop=(it == IT - 1 and kw == KS - 1),
                        )
            return P

        def evict_dve(P):
            # y0 = (P0+P1)+P2, y1 = (P1-P2)-P3. Engine split: ACT copies
            # P1/P2 off PSUM, DVE does the two remaining PSUM-coupled ops,
            # GpSimd the two all-SBUF ops. Frees banks promptly, no dn wait.
            Pv = [p.rearrange("p (a b) -> p a b", b=W) for p in P]
            og = opool.tile([128, 2, TH, W], FP32, name="og", tag="og")
            c1 = spool.tile([128, TH, W], FP32, name="c1", tag="c1")
            c2 = spool.tile([128, TH, W], FP32, name="c2", tag="c2")
            u = spool.tile([128, TH, W], FP32, name="u", tag="u")
            v = spool.tile([128, TH, W], FP32, name="v", tag="v")
            nc.scalar.copy(out=c1, in_=Pv[1])
            nc.scalar.copy(out=c2, in_=Pv[2])
            nc.vector.tensor_add(u, c1, Pv[0])
            nc.gpsimd.tensor_sub(v, c1, c2)
            nc.gpsimd.tensor_add(og[:, 0], u, c2)
            nc.vector.tensor_sub(og[:, 1], v, Pv[3])
            return og

        def evict_fin(og, g: int, ot: int, h: int):
            # demodulation scale (in-place, ACT) + store; even/odd output
            # rows are separate DMAs (strided DRAM rows via the j view)
            nc.scalar.mul(out=og, in_=og, mul=dn[ot])
            yv = y_d[ot * 128:(ot + 1) * 128].rearrange(
                "o (r j) w -> o r j w", j=2
            )
            r0 = g * (RG // 2) + h * TH
            for j in range(2):
                nc.sync.dma_start(
                    out=yv[:, r0:r0 + TH, j, :], in_=og[:, j]
                )

        def block(g: int, ot: int, h: int):
            P = half_matmuls(g, ot, h)
            og = evict_dve(P)
            evict_fin(og, g, ot, h)

        # group 0: first two blocks' matmuls + bank-freeing DVE evictions,
        # then denom (wsq ready by then), then the deferred ACT demod+store
        # (which waits on dn — emitted after sqrt to keep ACT's FIFO acyclic).
        p00 = half_matmuls(0, 0, 0)
        og00 = evict_dve(p00)
        p01 = half_matmuls(0, 0, 1)
        og01 = evict_dve(p01)
        emit_denom()
        evict_fin(og00, 0, 0, 0)
        evict_fin(og01, 0, 0, 1)
        lg_cast(2)
        for h in range(2):
            block(0, 1, h)

        for g in range(1, G):
            if g + 2 < G:
                lg_dma(g + 2)
            if g + 1 < G:
                for it in range(IT):
                    transform_it(g + 1, it)
            block(g, 0, 0)
            block(g, 0, 1)
            block(g, 1, 0)
            if g + 2 < G:
                lg_cast(g + 2)
            block(g, 1, 1)
    nc.finalize()
    return nc


_CACHE: dict = {}


def _get_nc() -> bass.Bass:
    if "nc" not in _CACHE:
        _CACHE["nc"] = build_nc()
    return _CACHE["nc"]


def make_in_maps(x, w, weight, affine_w, affine_b):
    x = np.ascontiguousarray(x, dtype=np.float32)
    w = np.ascontiguousarray(w, dtype=np.float32)
    weight = np.ascontiguousarray(weight, dtype=np.float32)
    affine_w = np.ascontiguousarray(affine_w, dtype=np.float32)
    affine_b = np.ascontiguousarray(affine_b, dtype=np.float32)
    return [
        {
            "x": x[c],
            "w": w[c],
            "weight": weight,
            "affine_w": affine_w,
            "affine_b": affine_b,
        }
        for c in range(B)
    ]


def run_on_hw(inputs: dict, trace: bool = False, tmpdir: str | None = None):
    from concourse.bass_utils import run_bass_kernel_spmd

    nc = _get_nc()
    in_maps = make_in_maps(**inputs)
    res = run_bass_kernel_spmd(
        nc, in_maps, core_ids=list(range(B)), trace=trace, tmpdir=tmpdir
    )
    y = np.stack([r["y"] for r in res.results], axis=0)
    return y, res


def kernel(x, w, weight, affine_w, affine_b):
    y, _ = run_on_hw(
        dict(x=x, w=w, weight=weight, affine_w=affine_w, affine_b=affine_b)
    )
    return y
